# revision 6
# baseline (speedup 1.0000x reference)
"""BinaryLinear (sign(x) @ sign(W).T + bias) on 8 trn2 NeuronCores.

Reference semantics (fp32):
    bw = where(W > 0, 1, -1); bx = where(x > 0, 1, -1)
    y  = bx @ bw.T + bias          x:[B,IN] W:[OUT,IN] bias:[OUT] y:[B,OUT]

Sharding: 2D mesh 4x2 — 4 batch shards x 2 out-feature shards. Each core:
    x_s:[2048,4096] w_s:[2048,4096] bias_s:[2048] -> y_s:[2048,2048]

Per-core kernel pipeline (values are +-1 so bf16/fp8 are exact; PSUM
accumulates fp32, and |sum| <= 4096 < 2^24 so results are exact integers):
  A) binarize: ACT Sign fp32 -> bf16, DMA to DRAM scratch in k-chunked
     layout [K_TILES, M, 512] (contiguous rows for the xbar transpose)
  B) transpose: xbar dma_start_transpose per (k-chunk, m-chunk) -> bf16
     [128, 4, mc] in SBUF, cast to fp8e4 caches kxm=[128,32,2048] (bx.T)
     and kxn=[128,32,2048] (bw.T), both SBUF-resident (8 MiB each)
  C) matmul: composable_matmul_tile_kernel over the caches; fp8 DoubleRow
     (contraction 256/matmul); bias added during PSUM->SBUF eviction.
"""

import numpy as np

import concourse.bass as bass
import concourse.tile as tile
from concourse import bacc, mybir
from concourse.bass import ds, ts
from concourse.bass_utils import run_bass_kernel_spmd
from concourse.kernels.tile_matmul import (
    ShapeInfo,
    TileKxM,
    TileKxN,
    TileMxN,
    composable_matmul_tile_kernel,
)

P = 128
B, IN, OUT = 8192, 4096, 4096
MESH_B, MESH_O = 4, 2  # 4 batch shards x 2 out shards = 8 cores
BS, OS = B // MESH_B, OUT // MESH_O  # per-core shard: 2048, 2048

F32 = mybir.dt.float32
BF16 = mybir.dt.bfloat16
FP8 = mybir.dt.float8e4


def build_binary_linear(Bs: int, In: int, Os: int):
    """Build the per-core bass program for x:[Bs,In] w:[Os,In] bias:[1,Os]."""
    KCH = 512  # k-chunk (columns per xbar transpose, = K_TILE)
    K_TILES = In // KCH  # 8
    KSUB = KCH // P  # 4
    MC = min(512, Bs)  # m-chunk for phase B transposes
    AKH = min(2048, In)  # phase-A tile width (k) to bound SBUF

    nc = bacc.Bacc(None, target_bir_lowering=False, debug=False)
    x = nc.dram_tensor("x", [Bs, In], F32, kind="ExternalInput")
    w = nc.dram_tensor("w", [Os, In], F32, kind="ExternalInput")
    bias = nc.dram_tensor("bias", [1, Os], F32, kind="ExternalInput")
    y = nc.dram_tensor("y", [Bs, Os], F32, kind="ExternalOutput")

    with tile.TileContext(nc) as tc:
        with (
            tc.tile_pool(name="dram", bufs=1, space="DRAM") as dram,
            tc.tile_pool(name="const", bufs=1) as const,
            tc.tile_pool(name="cache", bufs=1) as cache,
            tc.tile_pool(name="a_in", bufs=2) as a_in,
            tc.tile_pool(name="a_out", bufs=2) as a_out,
            tc.tile_pool(name="b_tmp", bufs=3) as b_tmp,
        ):
            # bias broadcast [P, Os] (DMA replicates the single DRAM row)
            bias_sb = const.tile([P, Os], F32)
            nc.sync.dma_start(bias_sb[:], bias[0:1, :].to_broadcast((P, Os)))

            # fp8 transposed caches, SBUF-resident
            kxm_cache = cache.tile([P, In // P, Bs], FP8)  # bx.T
            kxn_cache = cache.tile([P, In // P, Os], FP8)  # bw.T

            # bf16 scratch in DRAM, k-chunk-major: [K_TILES, M, KCH]
            x_scr = dram.tile([K_TILES, Bs, KCH], BF16)
            w_scr = dram.tile([K_TILES, Os, KCH], BF16)

            # ---- Phase A: binarize to bf16 scratch ----------------------
            # (v is_gt 0) - 0.5 -> {+0.5, -0.5}; exact for v == 0 too
            # (reference maps 0 -> -1). The 2x scale per operand is undone
            # by the *4 in the PSUM eviction.
            def binarize(src, scr, M, eng):
                for m0 in range(0, M, P):
                    for k0 in range(0, In, AKH):
                        t_in = a_in.tile([P, AKH], F32, tag="a_in")
                        nc.sync.dma_start(t_in[:], src[ds(m0, P), ds(k0, AKH)])
                        t_bin = a_out.tile([P, AKH], BF16, tag="a_out")
                        eng.tensor_scalar(
                            t_bin[:],
                            t_in[:],
                            0.0,
                            0.5,
                            mybir.AluOpType.is_gt,
                            mybir.AluOpType.subtract,
                        )
                        for kc in range(k0 // KCH, (k0 + AKH) // KCH):
                            nc.sync.dma_start(
                                scr[kc, ds(m0, P), :],
                                t_bin[:, ds(kc * KCH - k0, KCH)],
                            )

            # interleave x/w so both caches start filling early
            binarize(x, x_scr, Bs, nc.vector)
            binarize(w, w_scr, Os, nc.gpsimd)

            # ---- Phase B: xbar transpose + fp8 cast ---------------------
            def build_cache(scr, cch, M, cast_engine):
                for mc0 in range(0, M, MC):
                    for kc in range(K_TILES):
                        t = b_tmp.tile([P, KSUB, MC], BF16, tag="b_tmp")
                        nc.sync.dma_start_transpose(
                            t[:], scr[kc, ds(mc0, MC), :]
                        )
                        cast_engine.copy(cch[:, ts(kc, KSUB), ds(mc0, MC)], t[:])

            build_cache(x_scr, kxm_cache, Bs, nc.scalar)
            build_cache(w_scr, kxn_cache, Os, nc.scalar)

            # ---- Phase C: fp8 DoubleRow matmul + bias -------------------
            def kxm_producer(nc_, md: TileKxM):
                return kxm_cache[
                    :, ts(md.k_tile_idx, md.k_subtiles), ts(md.m_tile_idx, md.m_tile)
                ]

            def kxn_producer(nc_, md: TileKxN):
                return kxn_cache[
                    :, ts(md.k_tile_idx, md.k_subtiles), ts(md.n_tile_idx, md.n_tile)
                ]

            y3 = y.rearrange("(po pi) f -> pi po f", pi=P)

            def bias_reducer(nc_, psum, sbuf, md: TileMxN):
                # operands are +-0.5, so psum = y_int / 4
                n0 = md.n_tile_idx * md.n_tile + md.n_subtile_idx * md.n_subtile
                nc_.vector.scalar_tensor_tensor(
                    out=sbuf[:, 0, :],
                    in0=psum[:, : md.n_slice_size],
                    scalar=4.0,
                    in1=bias_sb[:, ds(n0, md.n_slice_size)],
                    op0=mybir.AluOpType.mult,
                    op1=mybir.AluOpType.add,
                )

            def y_consumer(nc_, mxn_tile, md: TileMxN):
                nc_.sync.dma_start(
                    y3[
                        :,
                        ts(md.m_tile_idx, md.m_subtiles),
                        ds(md.n_tile_idx * md.n_tile, md.n_slice_size),
                    ],
                    mxn_tile[:, :, : md.n_slice_size],
                )

            composable_matmul_tile_kernel(
                tc,
                kxm_shape=ShapeInfo(pdims=((P, In // P),), fdims=(Bs,)),
                kxn_shape=ShapeInfo(pdims=((P, In // P),), fdims=(Os,)),
                output_type=F32,
                kxm_producer=kxm_producer,
                kxn_producer=kxn_producer,
                mxn_consumer=y_consumer,
                mxn_subtile_reducer=bias_reducer,
                MATMUL_FREE_DIM=512,
                MAX_TILE_SIZE=512,
                MAX_K_TILE_SIZE=KCH,
                cache_tiles=False,
                temps_n_bufs=2,
                psum_n_bufs=2,
            )

    nc.compile()
    return nc


_NC_CACHE = {}


def _get_nc(Bs, In, Os):
    key = (Bs, In, Os)
    if key not in _NC_CACHE:
        _NC_CACHE[key] = build_binary_linear(Bs, In, Os)
    return _NC_CACHE[key]


def kernel(x: np.ndarray, weight: np.ndarray, bias: np.ndarray) -> np.ndarray:
    assert x.shape == (B, IN) and weight.shape == (OUT, IN) and bias.shape == (OUT,)
    nc = _get_nc(BS, IN, OS)

    in_maps = []
    for c in range(8):
        bi, oi = divmod(c, MESH_O)
        in_maps.append(
            {
                "x": np.ascontiguousarray(x[bi * BS : (bi + 1) * BS]),
                "w": np.ascontiguousarray(weight[oi * OS : (oi + 1) * OS]),
                "bias": np.ascontiguousarray(bias[oi * OS : (oi + 1) * OS])[None, :],
            }
        )

    r = run_bass_kernel_spmd(nc, in_maps, core_ids=list(range(8)))

    out = np.empty((B, OUT), dtype=np.float32)
    for c in range(8):
        bi, oi = divmod(c, MESH_O)
        out[bi * BS : (bi + 1) * BS, oi * OS : (oi + 1) * OS] = r.results[c]["y"]
    return out


# revision 7
# speedup vs baseline: 2.0013x; 2.0013x over previous
"""BinaryLinear (sign(x) @ sign(W).T + bias) on 8 trn2 NeuronCores.

Reference semantics (fp32):
    bw = where(W > 0, 1, -1); bx = where(x > 0, 1, -1)
    y  = bx @ bw.T + bias          x:[B,IN] W:[OUT,IN] bias:[OUT] y:[B,OUT]

Sharding: 2D mesh 4x2 — 4 batch shards x 2 out-feature shards. Each core:
    x_s:[2048,4096] w_s:[2048,4096] bias_s:[2048] -> y_s:[2048,2048]

Per-core kernel pipeline (values are +-1 so bf16/fp8 are exact; PSUM
accumulates fp32, and |sum| <= 4096 < 2^24 so results are exact integers):
  A) binarize: ACT Sign fp32 -> bf16, DMA to DRAM scratch in k-chunked
     layout [K_TILES, M, 512] (contiguous rows for the xbar transpose)
  B) transpose: xbar dma_start_transpose per (k-chunk, m-chunk) -> bf16
     [128, 4, mc] in SBUF, cast to fp8e4 caches kxm=[128,32,2048] (bx.T)
     and kxn=[128,32,2048] (bw.T), both SBUF-resident (8 MiB each)
  C) matmul: composable_matmul_tile_kernel over the caches; fp8 DoubleRow
     (contraction 256/matmul); bias added during PSUM->SBUF eviction.
"""

import numpy as np

import concourse.bass as bass
import concourse.tile as tile
from concourse import bacc, mybir
from concourse.bass import ds, ts
from concourse.bass_utils import run_bass_kernel_spmd
from concourse.kernels.tile_matmul import (
    ShapeInfo,
    TileKxM,
    TileKxN,
    TileMxN,
    composable_matmul_tile_kernel,
)

P = 128
B, IN, OUT = 8192, 4096, 4096
MESH_B, MESH_O = 4, 2  # 4 batch shards x 2 out shards = 8 cores
BS, OS = B // MESH_B, OUT // MESH_O  # per-core shard: 2048, 2048

F32 = mybir.dt.float32
BF16 = mybir.dt.bfloat16
FP8 = mybir.dt.float8e4


def build_binary_linear(Bs: int, In: int, Os: int):
    """Build the per-core bass program for x:[Bs,In] w:[Os,In] bias:[1,Os]."""
    KCH = 512  # k-chunk (columns per xbar transpose, = K_TILE)
    K_TILES = In // KCH  # 8
    KSUB = KCH // P  # 4
    MC = min(512, Bs)  # m-chunk for phase B transposes
    AKH = min(2048, In)  # phase-A tile width (k) to bound SBUF

    nc = bacc.Bacc(None, target_bir_lowering=False, debug=False)
    x = nc.dram_tensor("x", [Bs, In], F32, kind="ExternalInput")
    w = nc.dram_tensor("w", [Os, In], F32, kind="ExternalInput")
    bias = nc.dram_tensor("bias", [1, Os], F32, kind="ExternalInput")
    y = nc.dram_tensor("y", [Bs, Os], F32, kind="ExternalOutput")

    with tile.TileContext(nc) as tc:
        with (
            tc.tile_pool(name="dram", bufs=1, space="DRAM") as dram,
            tc.tile_pool(name="const", bufs=1) as const,
            tc.tile_pool(name="cache", bufs=1) as cache,
            tc.tile_pool(name="a_in", bufs=2) as a_in,
            tc.tile_pool(name="a_out", bufs=2) as a_out,
            tc.tile_pool(name="b_tmp", bufs=3) as b_tmp,
        ):
            # bias broadcast [P, Os] (DMA replicates the single DRAM row)
            bias_sb = const.tile([P, Os], F32)
            nc.sync.dma_start(bias_sb[:], bias[0:1, :].to_broadcast((P, Os)))

            # fp8 transposed caches, SBUF-resident
            kxm_cache = cache.tile([P, In // P, Bs], FP8)  # bx.T
            kxn_cache = cache.tile([P, In // P, Os], FP8)  # bw.T

            # bf16 scratch in DRAM (natural layout; xbar reads are strided)
            x_scr = dram.tile([Bs, In], BF16)
            w_scr = dram.tile([Os, In], BF16)

            # ---- Phase A: binarize to bf16 scratch ----------------------
            # (v is_gt 0) - 0.5 -> {+0.5, -0.5}; exact for v == 0 too
            # (reference maps 0 -> -1). The 2x scale per operand is undone
            # by the *4 in the PSUM eviction. DVE only — GpSimd ALU is ~25x
            # slower at elementwise (measured 987us for 8.4M elems).
            def binarize_chunk(src, scr, mc0):
                for m0 in range(mc0, mc0 + MC, P):
                    for k0 in range(0, In, AKH):
                        t_in = a_in.tile([P, AKH], F32, tag="a_in")
                        nc.sync.dma_start(t_in[:], src[ds(m0, P), ds(k0, AKH)])
                        t_bin = a_out.tile([P, AKH], BF16, tag="a_out")
                        nc.vector.tensor_scalar(
                            t_bin[:],
                            t_in[:],
                            0.0,
                            0.5,
                            mybir.AluOpType.is_gt,
                            mybir.AluOpType.subtract,
                        )
                        nc.sync.dma_start(scr[ds(m0, P), ds(k0, AKH)], t_bin[:])

            # ---- Phase B: xbar transpose + fp8 cast ---------------------
            def cache_chunk(scr, cch, mc0):
                scr3 = scr.rearrange("m (ko ki) -> m ko ki", ki=P)
                for kc in range(K_TILES):
                    t = b_tmp.tile([P, KSUB, MC], BF16, tag="b_tmp")
                    nc.sync.dma_start_transpose(
                        t[:], scr3[ds(mc0, MC), ts(kc, KSUB)]
                    )
                    nc.scalar.copy(cch[:, ts(kc, KSUB), ds(mc0, MC)], t[:])

            # interleave x/w chunk-wise so both caches fill progressively
            # and the matmul phase can start after the first chunks land
            assert Bs == Os
            for mc0 in range(0, Bs, MC):
                binarize_chunk(x, x_scr, mc0)
                cache_chunk(x_scr, kxm_cache, mc0)
                binarize_chunk(w, w_scr, mc0)
                cache_chunk(w_scr, kxn_cache, mc0)

            # ---- Phase C: fp8 DoubleRow matmul + bias -------------------
            def kxm_producer(nc_, md: TileKxM):
                return kxm_cache[
                    :, ts(md.k_tile_idx, md.k_subtiles), ts(md.m_tile_idx, md.m_tile)
                ]

            def kxn_producer(nc_, md: TileKxN):
                return kxn_cache[
                    :, ts(md.k_tile_idx, md.k_subtiles), ts(md.n_tile_idx, md.n_tile)
                ]

            y3 = y.rearrange("(po pi) f -> pi po f", pi=P)

            def bias_reducer(nc_, psum, sbuf, md: TileMxN):
                # operands are +-0.5, so psum = y_int / 4
                n0 = md.n_tile_idx * md.n_tile + md.n_subtile_idx * md.n_subtile
                nc_.vector.scalar_tensor_tensor(
                    out=sbuf[:, 0, :],
                    in0=psum[:, : md.n_slice_size],
                    scalar=4.0,
                    in1=bias_sb[:, ds(n0, md.n_slice_size)],
                    op0=mybir.AluOpType.mult,
                    op1=mybir.AluOpType.add,
                )

            def y_consumer(nc_, mxn_tile, md: TileMxN):
                nc_.sync.dma_start(
                    y3[
                        :,
                        ts(md.m_tile_idx, md.m_subtiles),
                        ds(md.n_tile_idx * md.n_tile, md.n_slice_size),
                    ],
                    mxn_tile[:, :, : md.n_slice_size],
                )

            composable_matmul_tile_kernel(
                tc,
                kxm_shape=ShapeInfo(pdims=((P, In // P),), fdims=(Bs,)),
                kxn_shape=ShapeInfo(pdims=((P, In // P),), fdims=(Os,)),
                output_type=F32,
                kxm_producer=kxm_producer,
                kxn_producer=kxn_producer,
                mxn_consumer=y_consumer,
                mxn_subtile_reducer=bias_reducer,
                MATMUL_FREE_DIM=512,
                MAX_TILE_SIZE=512,
                MAX_K_TILE_SIZE=KCH,
                cache_tiles=False,
                temps_n_bufs=2,
                psum_n_bufs=2,
            )

    nc.compile()
    return nc


_NC_CACHE = {}


def _get_nc(Bs, In, Os):
    key = (Bs, In, Os)
    if key not in _NC_CACHE:
        _NC_CACHE[key] = build_binary_linear(Bs, In, Os)
    return _NC_CACHE[key]


def kernel(x: np.ndarray, weight: np.ndarray, bias: np.ndarray) -> np.ndarray:
    assert x.shape == (B, IN) and weight.shape == (OUT, IN) and bias.shape == (OUT,)
    nc = _get_nc(BS, IN, OS)

    in_maps = []
    for c in range(8):
        bi, oi = divmod(c, MESH_O)
        in_maps.append(
            {
                "x": np.ascontiguousarray(x[bi * BS : (bi + 1) * BS]),
                "w": np.ascontiguousarray(weight[oi * OS : (oi + 1) * OS]),
                "bias": np.ascontiguousarray(bias[oi * OS : (oi + 1) * OS])[None, :],
            }
        )

    r = run_bass_kernel_spmd(nc, in_maps, core_ids=list(range(8)))

    out = np.empty((B, OUT), dtype=np.float32)
    for c in range(8):
        bi, oi = divmod(c, MESH_O)
        out[bi * BS : (bi + 1) * BS, oi * OS : (oi + 1) * OS] = r.results[c]["y"]
    return out
